# revision 2
# baseline (speedup 1.0000x reference)
"""Trainium2 Bass kernel for BasicAttention (depthwise-separable QKV conv + MHA).

Reference computation (fp32):
    x [4, 256, 64, 64] --depthwise 3x3 (pad 1)--> dw
    qkv = w_pw @ dw  (1x1 pointwise, 256 -> 768)
    4 heads x 64 dim attention over S = 64*64 = 4096 tokens per batch
    out [4, 256, 64, 64]

Sharding: 8 cores, core i handles batch b = i//2 and head-pair (0,1) or (2,3).
Each core computes the depthwise conv for its full batch (256 channels), the
pointwise projection only for its two heads' q/k/v rows, then attention.

Device-side layout tricks:
  * depthwise conv = 9 accumulating PE matmuls with diagonal weight matrices
    over shifted views of a zero-padded x held in SBUF
  * pointwise matmuls emit q^T/k^T packed as [qA^T;qB^T] (64+64 partitions) so
    the dots matmuls of the two heads land on disjoint PE row groups and run
    concurrently
  * attention is computed transposed: dots^T[t,s] = k^T(stationary) x q^T, so
    softmax exp (ScalarE) writes P^T straight into the stationary-operand
    layout that the attn@v matmul needs -- no big transposes
  * row sums of P come free as a 65th ones-column appended to v
  * logits are provably tiny (|logit| < ~0.5) so softmax skips max-subtraction
All matmuls bf16 with fp32 PSUM accumulation.
"""

import os
import sys

import numpy as np


def _ensure_imports():
    try:
        import concourse.bass  # noqa: F401
        return
    except ImportError:
        pass
    for p in (
        "/root/.axon_site",
        "/root/.axon_site/_ro/trn_rl_repo",
        "/root/.axon_site/_ro/pypackages",
        "/opt/trn_rl_repo",
        "/opt/pypackages",
    ):
        if os.path.isdir(p) and p not in sys.path:
            sys.path.append(p)
    import concourse.bass  # noqa: F401


B, C, H, W = 4, 256, 64, 64
S = H * W                     # 4096
HEADS, DH = 4, 64
SCALE = DH ** -0.5
NCORES = 8
PADW = W + 2                  # 66
SCHUNK = 512                  # s-columns processed per attention chunk
NCHUNK = S // SCHUNK          # 8
TBLK = 128                    # keys per t-block
NT = S // TBLK                # 32
NSB = SCHUNK // 128           # s-blocks of 128 rows per chunk

_compiled = None


def _build_program():
    import concourse.bacc as bacc
    import concourse.mybir as mybir
    import concourse.tile as tile

    bf16 = mybir.dt.bfloat16
    f32 = mybir.dt.float32
    Exp = mybir.ActivationFunctionType.Exp

    nc = bacc.Bacc("TRN2", target_bir_lowering=False, debug=False,
                   num_devices=NCORES)

    xb = nc.dram_tensor("xb", [2, 128, H, W], bf16, kind="ExternalInput")
    wdiag = nc.dram_tensor("wdiag", [2, 128, 9 * 128], bf16, kind="ExternalInput")
    wq = nc.dram_tensor("wq", [2, 128, 128], bf16, kind="ExternalInput")
    wk = nc.dram_tensor("wk", [2, 128, 128], bf16, kind="ExternalInput")
    wv = nc.dram_tensor("wv", [2, 128, 128], bf16, kind="ExternalInput")
    ident = nc.dram_tensor("ident", [128, 128], bf16, kind="ExternalInput")
    out = nc.dram_tensor("out", [2, S, DH], f32, kind="ExternalOutput")

    with tile.TileContext(nc) as tc:
        with (
            tc.tile_pool(name="persist", bufs=1) as pers,
            tc.tile_pool(name="psb", bufs=2) as ppool,
            tc.tile_pool(name="fin", bufs=4) as fin,
        ):
            # ---------------- persistent SBUF tiles ----------------
            qT_sb = pers.tile([128, S], bf16)     # [qA^T(64p); qB^T(64p)]
            kT_sb = pers.tile([128, S], bf16)
            vextA = pers.tile([128, NT * 65], bf16)  # per t-block: [v | 1]
            vextB = pers.tile([128, NT * 65], bf16)
            ident_sb = pers.tile([128, 128], bf16)
            nc.sync.dma_start(out=ident_sb[:], in_=ident[:])
            nc.gpsimd.memset(vextA[:], 1.0)
            nc.gpsimd.memset(vextB[:], 1.0)

            # ---------------- preamble: conv + pointwise + v ----------------
            with (
                tc.tile_pool(name="pre", bufs=1) as pre,
                tc.tile_pool(name="pre_ps", bufs=2, space="PSUM") as pre_ps,
            ):
                xpad = []
                wdiag_sb = []
                dw_sb = []
                for g in range(2):
                    xp = pre.tile([128, PADW * PADW], bf16, name=f"xpad{g}")
                    nc.gpsimd.memset(xp[:], 0.0)
                    xp3 = xp.rearrange("p (h w) -> p h w", h=PADW)
                    nc.sync.dma_start(out=xp3[:, 1:65, 1:65], in_=xb[g])
                    xpad.append(xp3)
                    wd = pre.tile([128, 9 * 128], bf16, name=f"wdiag{g}")
                    nc.sync.dma_start(out=wd[:], in_=wdiag[g])
                    wdiag_sb.append(wd)
                    dw_sb.append(pre.tile([128, S], bf16, name=f"dw{g}"))

                wq_sb = pre.tile([128, 256], bf16)
                wk_sb = pre.tile([128, 256], bf16)
                wv_sb = pre.tile([128, 256], bf16)
                for kg in range(2):
                    nc.sync.dma_start(out=wq_sb[:, kg * 128:(kg + 1) * 128], in_=wq[kg])
                    nc.sync.dma_start(out=wk_sb[:, kg * 128:(kg + 1) * 128], in_=wk[kg])
                    nc.sync.dma_start(out=wv_sb[:, kg * 128:(kg + 1) * 128], in_=wv[kg])

                # depthwise conv: out[c, h*64+w] = sum_tap wd[c,tap] * xpad[c, h+dy, w+dx]
                for g in range(2):
                    for ch in range(8):           # 8 h-rows -> 512 outputs
                        h0 = ch * 8
                        cps = pre_ps.tile([128, 512], f32, tag="convps")
                        t = 0
                        for dy in range(3):
                            for dx in range(3):
                                nc.tensor.matmul(
                                    cps[:],
                                    lhsT=wdiag_sb[g][:, t * 128:(t + 1) * 128],
                                    rhs=xpad[g][:, h0 + dy:h0 + dy + 8, dx:dx + 64],
                                    start=(t == 0), stop=(t == 8),
                                )
                                t += 1
                        nc.vector.tensor_copy(
                            dw_sb[g][:, ch * 512:(ch + 1) * 512], cps[:])

                # pointwise: qT/kT/vT (each [128, S]; partitions = packed heads)
                vT_sb = pre.tile([128, S], bf16)
                for dst, wsb in ((qT_sb, wq_sb), (kT_sb, wk_sb), (vT_sb, wv_sb)):
                    for chn in range(NCHUNK):
                        pps = pre_ps.tile([128, 512], f32, tag="pwps")
                        for kg in range(2):
                            nc.tensor.matmul(
                                pps[:],
                                lhsT=wsb[:, kg * 128:(kg + 1) * 128],
                                rhs=dw_sb[kg][:, chn * 512:(chn + 1) * 512],
                                start=(kg == 0), stop=(kg == 1),
                            )
                        nc.scalar.copy(dst[:, chn * 512:(chn + 1) * 512], pps[:])

                # transpose v^T -> v tiles [t, d] with ones column appended
                for t in range(NT):
                    tps = pre_ps.tile([128, 128], bf16, tag="vtps")
                    nc.tensor.transpose(
                        tps[:], vT_sb[:, t * 128:(t + 1) * 128], ident_sb[:])
                    nc.vector.tensor_copy(
                        vextA[:, t * 65:t * 65 + 64], tps[:, 0:64])
                    nc.vector.tensor_copy(
                        vextB[:, t * 65:t * 65 + 64], tps[:, 64:128])

            # ---------------- attention ----------------
            # quad sizes: 10x3 + 1x2 t-blocks (32 total) per s-chunk
            quads = [(3 * q, 3) for q in range(10)] + [(30, 2)]

            with (
                tc.tile_pool(name="qpsA", bufs=1, space="PSUM") as qpsA,
                tc.tile_pool(name="qpsB", bufs=1, space="PSUM") as qpsB,
                tc.tile_pool(name="avps", bufs=2, space="PSUM") as avps,
            ):
                pbuf = {}

                def emit_av(c):
                    for h in range(2):
                        vext = vextA if h == 0 else vextB
                        ptile = pbuf[h]
                        for sb in range(NSB):
                            avp = avps.tile([128, 65], f32, name="avp")
                            for t in range(NT):
                                nc.tensor.matmul(
                                    avp[:],
                                    lhsT=ptile[:, t * SCHUNK + sb * 128:
                                               t * SCHUNK + sb * 128 + 128],
                                    rhs=vext[:, t * 65:(t + 1) * 65],
                                    start=(t == 0), stop=(t == NT - 1),
                                )
                            rec = fin.tile([128, 1], f32, tag="rec", name="rec")
                            nc.vector.reciprocal(rec[:], avp[:, 64:65])
                            osb = fin.tile([128, 64], f32, tag="osb", name="osb")
                            nc.vector.tensor_scalar_mul(osb[:], avp[:, 0:64], rec[:])
                            nc.sync.dma_start(
                                out=out[h, c * SCHUNK + sb * 128:
                                        c * SCHUNK + sb * 128 + 128, :],
                                in_=osb[:])

                for c in range(NCHUNK):
                    newp = [
                        ppool.tile([128, NT * SCHUNK], bf16, tag=f"P{h}",
                                   name=f"P{h}")
                        for h in range(2)
                    ]
                    for (t0, qn) in quads:
                        qp = [
                            qpsA.tile([128, 3 * 512], f32, name="qpa"),
                            qpsB.tile([128, 3 * 512], f32, name="qpb"),
                        ]
                        for j in range(qn):
                            t = t0 + j
                            for h in range(2):
                                nc.tensor.matmul(
                                    qp[h][:, j * 512:(j + 1) * 512],
                                    lhsT=kT_sb[h * 64:(h + 1) * 64,
                                               t * 128:(t + 1) * 128],
                                    rhs=qT_sb[h * 64:(h + 1) * 64,
                                              c * SCHUNK:(c + 1) * SCHUNK],
                                    start=True, stop=True,
                                )
                        for h in range(2):
                            nc.scalar.activation(
                                newp[h][:, t0 * 512:(t0 + qn) * 512],
                                qp[h][:, 0:qn * 512],
                                Exp, scale=SCALE)
                    if c > 0:
                        emit_av(c - 1)
                    pbuf[0], pbuf[1] = newp
                emit_av(NCHUNK - 1)

    nc.compile()
    return nc


def _get_compiled():
    global _compiled
    if _compiled is None:
        _ensure_imports()
        _compiled = _build_program()
    return _compiled


def _prep_core_inputs(x, w_dw, w_pw, core):
    import ml_dtypes
    bf16 = ml_dtypes.bfloat16

    b = core // 2
    hA = 2 * (core % 2)
    hB = hA + 1

    xb = np.ascontiguousarray(x[b].reshape(2, 128, H, W)).astype(bf16)

    wd = np.zeros((2, 128, 9, 128), np.float32)
    taps = w_dw[:, 0].reshape(C, 9)          # [c, tap]
    for g in range(2):
        for t in range(9):
            np.fill_diagonal(wd[g, :, t, :], taps[g * 128:(g + 1) * 128, t])
    wdiag = wd.reshape(2, 128, 9 * 128).astype(bf16)

    def pack(base):
        # [256 c, 128] with cols 0:64 = head A rows, 64:128 = head B rows
        rows = np.concatenate([
            w_pw[base + hA * 64: base + hA * 64 + 64, :],
            w_pw[base + hB * 64: base + hB * 64 + 64, :],
        ], axis=0)                            # [128, 256]
        m = rows.T.reshape(2, 128, 128)       # [kg, c_part, o]
        return np.ascontiguousarray(m).astype(bf16)

    return {
        "xb": xb,
        "wdiag": wdiag,
        "wq": pack(0),
        "wk": pack(C),
        "wv": pack(2 * C),
        "ident": np.eye(128, dtype=bf16),
    }


def kernel(x, w_dw, w_pw, _trace=False, _tmpdir=None):
    _ensure_imports()
    from concourse.bass_utils import run_bass_kernel_spmd

    nc = _get_compiled()
    in_maps = [_prep_core_inputs(x, w_dw, w_pw, i) for i in range(NCORES)]
    res = run_bass_kernel_spmd(nc, in_maps, list(range(NCORES)),
                               trace=_trace, tmpdir=_tmpdir)

    full = np.empty((B, C, H, W), np.float32)
    for i in range(NCORES):
        b = i // 2
        oc = res.results[i]["out"]            # [2, S, DH]
        for j in range(2):
            h = 2 * (i % 2) + j
            full[b, h * 64:(h + 1) * 64] = oc[j].T.reshape(DH, H, W)
    if _trace:
        return full, res
    return full
